# revision 1
# baseline (speedup 1.0000x reference)
"""Trainium2 Bass kernel for nn_InternalMAFE_59270548684863.

Key facts (hardcoded from the problem):
  - Output depends ONLY on branch 1 (p=7, n=288) of the reference; the
    n2=1008 branch feeds a dead projection and is never computed.
  - out = o1 @ proj_len_w.T + proj_len_b,  o1 = branch(x, 7, h1, w_k1, w_v1, ...)
  - Softmax normalizes over the batch axis, so we batch-shard (512 rows/core)
    and AllReduce the per-(slice, feature) exp-sums (a [128,24] f32 buffer).
    Constant-shift softmax (exp(s*scale - 50)) avoids a cross-core max pass.
  - s = h1 @ (x_i w_k)^T is fused as W_hk = h1 @ w_k^T (one 288^3 product)
    so each slice needs only ONE matmul chain for the logits.
  - All matmuls run in bf16 (fp32 matmul is a 2-pass LOW_HIGH on trn2 PE);
    PSUM accumulation, softmax and the gated scan stay fp32.
  - Schedule: all logit/exp work first -> AllReduce fires mid-kernel and is
    hidden behind the v-matmuls and the proj_len_w de-interleave transposes.
"""

import math

import numpy as np

import concourse.bacc as bacc
import concourse.masks as masks
import concourse.mybir as mybir
import concourse.tile as tile
from concourse.bass_utils import run_bass_kernel_spmd

N_CORES = 8
B = 4096
BL = B // N_CORES  # 512 rows per core
INP = 2016
P1 = 7
N1 = 288
SEQ = 1024
SCALE = 1.0 / math.sqrt(N1)
SHIFT = -50.0
F32 = mybir.dt.float32
BF16 = mybir.dt.bfloat16
CH = [(0, 128), (128, 128), (256, 32)]
AF = mybir.ActivationFunctionType


def build():
    nc = bacc.Bacc(
        "TRN2", target_bir_lowering=False, debug=False, num_devices=N_CORES
    )
    x = nc.dram_tensor("x", [BL, INP], F32, kind="ExternalInput").ap()
    wk = nc.dram_tensor("w_k1", [N1, N1], F32, kind="ExternalInput").ap()
    wv = nc.dram_tensor("w_v1", [N1, N1], F32, kind="ExternalInput").ap()
    h1 = nc.dram_tensor("h1", [N1, N1], F32, kind="ExternalInput").ap()
    a1 = nc.dram_tensor("alpha1", [1], F32, kind="ExternalInput").ap()
    a2 = nc.dram_tensor("alpha2", [1], F32, kind="ExternalInput").ap()
    b1 = nc.dram_tensor("beta1", [1], F32, kind="ExternalInput").ap()
    b2 = nc.dram_tensor("beta2", [1], F32, kind="ExternalInput").ap()
    plw = nc.dram_tensor("proj_len_w", [SEQ, INP], F32, kind="ExternalInput").ap()
    plb = nc.dram_tensor("proj_len_b", [SEQ], F32, kind="ExternalInput").ap()
    out = nc.dram_tensor("out", [BL, SEQ], F32, kind="ExternalOutput").ap()

    with tile.TileContext(nc) as tc:
        with (
            tc.tile_pool(name="const", bufs=1) as cpool,
            tc.tile_pool(name="plwn", bufs=1) as plwpool,
            tc.tile_pool(name="plwb", bufs=4) as plwbpool,
            tc.tile_pool(name="rk", bufs=1) as rkpool,
            tc.tile_pool(name="dram", bufs=1, space="DRAM") as dpool,
        ):
            # ---------------- constants ----------------
            ident = cpool.tile([128, 128], BF16, tag="ident", name="ident")
            masks.make_identity(nc, ident[:])
            ones = cpool.tile([1, 128], BF16, tag="ones", name="ones")
            nc.vector.memset(ones[:], 1.0)

            scal = cpool.tile([1, 4], F32, tag="scal", name="scal")
            for idx, ap in enumerate((a1, a2, b1, b2)):
                nc.sync.dma_start(scal[0:1, idx : idx + 1], ap[:])

            plb_sb = cpool.tile([1, SEQ], BF16, tag="plb", name="plb")
            plb_f = cpool.tile([1, SEQ], F32, tag="plb_f", name="plb_f")
            nc.sync.dma_start(plb_f[:], plb[:])
            nc.vector.tensor_copy(plb_sb[:], plb_f[:])

            densb = cpool.tile([128, 24], F32, tag="densb", name="densb")
            nc.vector.memset(densb[:], 0.0)
            shiftc = cpool.tile([128, 1], F32, tag="shiftc", name="shiftc")
            nc.vector.memset(shiftc[:], SHIFT)
            den_all = cpool.tile([128, 24], F32, tag="den_all", name="den_all")
            recip = cpool.tile([128, 24], F32, tag="recip", name="recip")

            cc_in = dpool.tile([128, 24], F32)
            cc_out = dpool.tile([128, 24], F32, addr_space="Shared")

            # fp32 scan state (bf16 mirrors are allocated in phase C)
            ys = [
                [cpool.tile([cnt, BL], F32, tag=f"ys{i}_{c}", name=f"ys{i}_{c}") for c, (j0, cnt) in enumerate(CH)]
                for i in range(P1)
            ]

            # ---------------- phase A/B: weights, x, logits, AR, vT --------
            with (
                tc.tile_pool(name="xn", bufs=2) as xpool,
                tc.tile_pool(name="xnb", bufs=4) as xbpool,
                tc.tile_pool(name="xiT", bufs=1) as xtpool,
                tc.tile_pool(name="ee", bufs=1) as epool,
                tc.tile_pool(name="psT", bufs=2, space="PSUM") as psT,
                tc.tile_pool(name="psS", bufs=2, space="PSUM") as psS,
                tc.tile_pool(name="psV", bufs=2, space="PSUM") as psV,
            ):
                # broadcast the 4 gate scalars to all 128 partitions via PE
                onesf = cpool.tile([1, 128], F32, tag="onesf", name="onesf")
                nc.vector.memset(onesf[:], 1.0)
                pbc = psS.tile([128, 512], F32, tag="ps_st", name="ps_bc")
                nc.tensor.matmul(pbc[:, 0:4], onesf[:], scal[:], start=True, stop=True)
                bcast = cpool.tile([128, 4], F32, tag="bcast", name="bcast")
                nc.vector.tensor_copy(bcast[:], pbc[:, 0:4])

                # weights -> bf16
                wk_b, wv_b, h1_b = [], [], []
                for t, (m0, mc) in enumerate(CH):
                    wtf = xpool.tile([mc, N1], F32, tag="wtmp", name="wtmp", bufs=3)
                    nc.sync.dma_start(wtf[:], wk[m0 : m0 + mc, :])
                    wt = cpool.tile([mc, N1], BF16, tag=f"wkb{t}", name=f"wkb{t}")
                    nc.vector.tensor_copy(wt[:], wtf[:])
                    wk_b.append(wt)
                    vtf = xpool.tile([mc, N1], F32, tag="wtmp", name="wtmp", bufs=3)
                    nc.sync.dma_start(vtf[:], wv[m0 : m0 + mc, :])
                    vt = cpool.tile([mc, N1], BF16, tag=f"wvb{t}", name=f"wvb{t}")
                    nc.vector.tensor_copy(vt[:], vtf[:])
                    wv_b.append(vt)
                    htf = xpool.tile([mc, N1], F32, tag="wtmp", name="wtmp", bufs=3)
                    nc.sync.dma_start(htf[:], h1[m0 : m0 + mc, :])
                    ht = cpool.tile([mc, N1], BF16, tag=f"h1b{t}", name=f"h1b{t}")
                    nc.vector.tensor_copy(ht[:], htf[:])
                    h1_b.append(ht)

                # h1T[l, j] = h1[j, l] and wkT[l, m] = wk[m, l]  (bf16)
                h1T, wkT = [], []
                for lt, (l0, lc) in enumerate(CH):
                    ps = psT.tile([128, 512], BF16, tag="tp", name="tp")
                    for jt, (j0, jc) in enumerate(CH):
                        nc.tensor.transpose(
                            ps[0:lc, j0 : j0 + jc],
                            h1_b[jt][:, l0 : l0 + lc],
                            ident[0:jc, 0:jc],
                        )
                    hT = cpool.tile([lc, N1], BF16, tag=f"h1T{lt}", name=f"h1T{lt}")
                    nc.vector.tensor_copy(hT[:], ps[0:lc, 0:N1])
                    h1T.append(hT)
                    ps2 = psT.tile([128, 512], BF16, tag="tp", name="tp")
                    for mt, (m0, mc) in enumerate(CH):
                        nc.tensor.transpose(
                            ps2[0:lc, m0 : m0 + mc],
                            wk_b[mt][:, l0 : l0 + lc],
                            ident[0:mc, 0:mc],
                        )
                    wTl = cpool.tile([lc, N1], BF16, tag=f"wkT{lt}", name=f"wkT{lt}")
                    nc.vector.tensor_copy(wTl[:], ps2[0:lc, 0:N1])
                    wkT.append(wTl)

                # W_hkT[m, j] = sum_l wk[m,l] h1[j,l]: lhsT=wkT, rhs=h1T (K=l)
                whkT = []
                for mt, (m0, mc) in enumerate(CH):
                    pw = psS.tile([128, 512], F32, tag="ps_st", name="ps_whk")
                    for lt, (l0, lc) in enumerate(CH):
                        nc.tensor.matmul(
                            pw[0:mc, 0:N1],
                            wkT[lt][:, m0 : m0 + mc],
                            h1T[lt][:],
                            start=(lt == 0),
                            stop=(lt == 2),
                        )
                    wTt = cpool.tile([mc, N1], BF16, tag=f"whkT{mt}", name=f"whkT{mt}")
                    nc.vector.tensor_copy(wTt[:], pw[0:mc, 0:N1])
                    whkT.append(wTt)

                # x shard: fp32 load -> bf16 convert
                xnb = []
                for bt in range(4):
                    xt = xpool.tile([128, INP], F32, tag="xn", name="xn")
                    nc.sync.dma_start(xt[:], x[bt * 128 : (bt + 1) * 128, :])
                    xb = xbpool.tile([128, INP], BF16, tag="xnb", name="xnb")
                    nc.vector.tensor_copy(
                        xb[:].rearrange("p (i j) -> p i j", i=P1),
                        xt[:].rearrange("p (j i) -> p j i", i=P1).rearrange("p j i -> p i j"),
                    )
                    xnb.append(xb)

                # prefetch plw half-0 (DMA + bf16 cast) so its de-interleave
                # transposes are ready to fill the AllReduce window
                pw4_h0 = []
                for st in range(4):
                    pwt = plwpool.tile([128, INP], F32, tag="plwn", name="plwn")
                    nc.sync.dma_start(pwt[:], plw[st * 128 : (st + 1) * 128, :])
                    pwb = plwbpool.tile([128, INP], BF16, tag="plwb", name="plwb")
                    nc.vector.tensor_copy(
                        pwb[:].rearrange("p (i j) -> p i j", i=P1),
                        pwt[:].rearrange("p (j i) -> p j i", i=P1).rearrange("p j i -> p i j"),
                    )
                    pw4_h0.append(pwb)

                # all de-interleaving transposes + all logits/exp first so the
                # AllReduce can fire while vT / plw transposes run
                xiT = [[None] * 3 for _ in range(P1)]
                E = [[None] * 3 for _ in range(P1)]
                for i in range(P1):
                    for c, (j0, cnt) in enumerate(CH):
                        xi = xtpool.tile([cnt, BL], BF16, tag=f"xiT{i}_{c}", name=f"xiT{i}_{c}")
                        if False:
                            pass
                        else:
                            ps = psT.tile([128, 512], BF16, tag="tp", name="tp")
                            for bt in range(4):
                                s_ap = xnb[bt][:, i * N1 + j0 : i * N1 + j0 + cnt]
                                nc.tensor.transpose(
                                    ps[0:cnt, bt * 128 : (bt + 1) * 128],
                                    s_ap,
                                    ident[:],
                                )
                            nc.vector.tensor_copy(xi[:], ps[0:cnt, :])
                        xiT[i][c] = xi

                    for jt, (j0, jc) in enumerate(CH):
                        pst = psS.tile([128, 512], F32, tag="ps_st", name="ps_st")
                        for lt, (l0, lc) in enumerate(CH):
                            nc.tensor.matmul(
                                pst[0:jc, :],
                                whkT[lt][:, j0 : j0 + jc],
                                xiT[i][lt][:],
                                start=(lt == 0),
                                stop=(lt == 2),
                            )
                        ec = epool.tile([jc, BL], F32, tag=f"e{i}_{jt}", name=f"e{i}_{jt}")
                        col = i * 3 + jt
                        nc.scalar.activation(
                            ec[:],
                            pst[0:jc, :],
                            AF.Exp,
                            bias=shiftc[0:jc, 0:1],
                            scale=SCALE,
                            accum_out=densb[0:jc, col : col + 1],
                        )
                        E[i][jt] = ec

                # ---- AllReduce of exp-sums (overlaps vT + plw transposes) --
                nc.gpsimd.dma_start(cc_in[:], densb[:])
                nc.gpsimd.collective_compute(
                    "AllReduce",
                    mybir.AluOpType.add,
                    replica_groups=[list(range(N_CORES))],
                    ins=[cc_in[:]],
                    outs=[cc_out[:]],
                )

                # vT = (x_i @ wv)^T ; ys = vT * E (normalized later)
                for i in range(P1):
                    for ntc, (n0, ncnt) in enumerate(CH):
                        pv = psV.tile([128, 512], F32, tag="ps_vt", name="ps_vt")
                        for mt, (m0, mc) in enumerate(CH):
                            nc.tensor.matmul(
                                pv[0:ncnt, :],
                                wv_b[mt][:, n0 : n0 + ncnt],
                                xiT[i][mt][:],
                                start=(mt == 0),
                                stop=(mt == 2),
                            )
                        nc.vector.tensor_mul(ys[i][ntc][:], pv[0:ncnt, :], E[i][ntc][:])

            nc.gpsimd.dma_start(den_all[:], cc_out[:])
            nc.vector.reciprocal(recip[:], den_all[:])

            # ---------------- phase C: plw K-tiles, scan, projection -------
            with (
                tc.tile_pool(name="ysb", bufs=1) as ysbpool,
                tc.tile_pool(name="tmp", bufs=1) as tmppool,
                tc.tile_pool(name="osb", bufs=2) as outpool,
                tc.tile_pool(name="psT2", bufs=2, space="PSUM") as psT2,
                tc.tile_pool(name="psP", bufs=4, space="PSUM") as psP,
            ):
                ysb = [
                    [ysbpool.tile([cnt, BL], BF16, tag=f"ysb{i}_{c}", name=f"ysb{i}_{c}") for c, (j0, cnt) in enumerate(CH)]
                    for i in range(P1)
                ]
                rk_halves = [[[None] * 3 for _ in range(P1)] for _ in range(2)]
                scan_emitted = False
                for half in range(2):
                    # load 4 plw row-tiles, convert to bf16, de-interleave
                    if half == 0:
                        pw4 = pw4_h0
                    else:
                        pw4 = []
                        for st in range(4):
                            pwt = plwpool.tile([128, INP], F32, tag="plwn", name="plwn")
                            r0 = (half * 4 + st) * 128
                            nc.sync.dma_start(pwt[:], plw[r0 : r0 + 128, :])
                            pwb = plwbpool.tile([128, INP], BF16, tag="plwb", name="plwb")
                            nc.vector.tensor_copy(
                                pwb[:].rearrange("p (i j) -> p i j", i=P1),
                                pwt[:].rearrange("p (j i) -> p j i", i=P1).rearrange("p j i -> p i j"),
                            )
                            pw4.append(pwb)
                    rk = rk_halves[half]
                    for i in range(P1):
                        for c, (j0, cnt) in enumerate(CH):
                            rkt = rkpool.tile([cnt, 512], BF16, tag=f"rk{i}_{c}", name=f"rk{i}_{c}")
                            if False:
                                pass
                            else:
                                ps = psT2.tile([128, 512], BF16, tag="tp2", name="tp2")
                                for st in range(4):
                                    s_ap = pw4[st][:, i * N1 + j0 : i * N1 + j0 + cnt]
                                    nc.tensor.transpose(
                                        ps[0:cnt, st * 128 : (st + 1) * 128],
                                        s_ap,
                                        ident[:],
                                    )
                                nc.vector.tensor_copy(rkt[:], ps[0:cnt, :])
                            rk[i][c] = rkt

                    if not scan_emitted:
                        # normalize + gated scan; bf16 mirrors for projection
                        scan_emitted = True
                        for i in range(P1):
                            for c, (j0, cnt) in enumerate(CH):
                                col = i * 3 + c
                                nc.scalar.mul(
                                    ys[i][c][:],
                                    ys[i][c][:],
                                    mul=recip[0:cnt, col : col + 1],
                                )
                            if i >= 1:
                                for c, (j0, cnt) in enumerate(CH):
                                    tt = tmppool.tile([cnt, BL], F32, tag=f"tt{c}", name=f"tt{c}")
                                    ts = tmppool.tile([cnt, BL], F32, tag=f"ts{c}", name=f"ts{c}")
                                    nc.scalar.activation(
                                        tt[:],
                                        ys[i - 1][c][:],
                                        AF.Tanh,
                                        bias=bcast[0:cnt, 2:3],
                                        scale=bcast[0:cnt, 0:1],
                                    )
                                    nc.scalar.activation(
                                        ts[:],
                                        ys[i - 1][c][:],
                                        AF.Sigmoid,
                                        bias=bcast[0:cnt, 3:4],
                                        scale=bcast[0:cnt, 1:2],
                                    )
                                    nc.vector.tensor_mul(tt[:], tt[:], ts[:])
                                    nc.vector.tensor_add(
                                        ys[i][c][:], ys[i][c][:], tt[:]
                                    )
                            for c, (j0, cnt) in enumerate(CH):
                                if c % 2 == 0:
                                    nc.scalar.copy(ysb[i][c][:], ys[i][c][:])
                                else:
                                    nc.vector.tensor_copy(ysb[i][c][:], ys[i][c][:])

                    # projection for this s-half: 4 batch groups of 128
                    pps = []
                    for bc in range(4):
                        pp = psP.tile([128, 512], F32, tag="pj", name="pj")
                        nc.tensor.matmul(
                            pp[:],
                            ones[:],
                            plb_sb[0:1, half * 512 : (half + 1) * 512],
                            start=True,
                            stop=False,
                        )
                        pps.append(pp)
                    for i in range(P1):
                        for c, (j0, cnt) in enumerate(CH):
                            last = i == P1 - 1 and c == 2
                            for bc in range(4):
                                nc.tensor.matmul(
                                    pps[bc][:],
                                    ysb[i][c][:, bc * 128 : (bc + 1) * 128],
                                    rk[i][c][:],
                                    start=False,
                                    stop=last,
                                )
                    for bc in range(4):
                        ob = outpool.tile([128, 512], F32, tag="osb", name="osb")
                        nc.vector.tensor_copy(ob[:], pps[bc][:])
                        nc.sync.dma_start(
                            out[bc * 128 : (bc + 1) * 128, half * 512 : (half + 1) * 512],
                            ob[:],
                        )

    nc.compile()
    return nc


_NC = None


def _get_nc():
    global _NC
    if _NC is None:
        _NC = build()
    return _NC


def run(inputs, trace=False):
    nc = _get_nc()
    rep_keys = [
        "w_k1",
        "w_v1",
        "h1",
        "alpha1",
        "alpha2",
        "beta1",
        "beta2",
        "proj_len_w",
        "proj_len_b",
    ]
    x = np.ascontiguousarray(inputs["x"], dtype=np.float32)
    rep = {k: np.ascontiguousarray(inputs[k], dtype=np.float32) for k in rep_keys}
    in_maps = [
        {"x": x[c * BL : (c + 1) * BL], **rep} for c in range(N_CORES)
    ]
    res = run_bass_kernel_spmd(
        nc, in_maps, core_ids=list(range(N_CORES)), trace=trace
    )
    full = np.concatenate([res.results[c]["out"] for c in range(N_CORES)], axis=0)
    return full, res


def kernel(**inputs):
    full, _ = run(inputs, trace=False)
    return full



# revision 6
# speedup vs baseline: 1.0661x; 1.0661x over previous
"""Trainium2 Bass kernel for nn_InternalMAFE_59270548684863.

Key facts (hardcoded from the problem):
  - Output depends ONLY on branch 1 (p=7, n=288) of the reference; the
    n2=1008 branch feeds a dead projection and is never computed.
  - out = o1 @ proj_len_w.T + proj_len_b,  o1 = branch(x, 7, h1, w_k1, w_v1, ...)
  - Softmax normalizes over the batch axis, so we batch-shard (512 rows/core)
    and AllReduce the per-(slice, feature) exp-sums (a [128,24] f32 buffer).
    Constant-shift softmax (exp(s*scale - 50)) avoids a cross-core max pass.
  - s = h1 @ (x_i w_k)^T is fused as W_hk = h1 @ w_k^T (one 288^3 product)
    so each slice needs only ONE matmul chain for the logits.
  - All matmuls run in bf16; PSUM accumulation and the gated scan stay fp32.

v2 schedule (vs the 227us baseline):
  - fp32->bf16 casts are contiguous (fast DVE mode); the feature
    de-interleave (stride 7) happens inside the PE transposes via strided
    stationary-operand views instead of strided DVE copies.
  - The exp-sum AllReduce is issued as early as possible (only the logit
    path precedes it); the wait is hidden under vT matmuls and the entire
    proj_len_w load/cast/transpose pipeline.
  - softmax normalization is folded into the scan with
    scalar_tensor_tensor: y_i = (z_i * recip) + tanh(..)*sigmoid(..).
  - The gate product runs on the otherwise-idle GPSIMD engine.
  - The projection accumulates per-slice into 8 PSUM banks while the scan
    runs, with proj_len_b pre-loaded via a ones-outer-product matmul.
"""

import math

import numpy as np

import concourse.bacc as bacc
import concourse.masks as masks
import concourse.mybir as mybir
import concourse.tile as tile
from concourse.bass_utils import run_bass_kernel_spmd

N_CORES = 8
B = 4096
BL = B // N_CORES  # 512 rows per core
INP = 2016
P1 = 7
N1 = 288
SEQ = 1024
SCALE = 1.0 / math.sqrt(N1)
SHIFT = -50.0
F32 = mybir.dt.float32
BF16 = mybir.dt.bfloat16
CH = [(0, 128), (128, 128), (256, 32)]
AF = mybir.ActivationFunctionType
ALU = mybir.AluOpType


def build():
    nc = bacc.Bacc(
        "TRN2", target_bir_lowering=False, debug=False, num_devices=N_CORES
    )
    x = nc.dram_tensor("x", [BL, INP], F32, kind="ExternalInput").ap()
    wk = nc.dram_tensor("w_k1", [N1, N1], F32, kind="ExternalInput").ap()
    wv = nc.dram_tensor("w_v1", [N1, N1], F32, kind="ExternalInput").ap()
    h1 = nc.dram_tensor("h1", [N1, N1], F32, kind="ExternalInput").ap()
    a1 = nc.dram_tensor("alpha1", [1], F32, kind="ExternalInput").ap()
    a2 = nc.dram_tensor("alpha2", [1], F32, kind="ExternalInput").ap()
    b1 = nc.dram_tensor("beta1", [1], F32, kind="ExternalInput").ap()
    b2 = nc.dram_tensor("beta2", [1], F32, kind="ExternalInput").ap()
    plw = nc.dram_tensor("proj_len_w", [SEQ, INP], F32, kind="ExternalInput").ap()
    plb = nc.dram_tensor("proj_len_b", [SEQ], F32, kind="ExternalInput").ap()
    out = nc.dram_tensor("out", [BL, SEQ], F32, kind="ExternalOutput").ap()

    def deint(ap_2d, i, j0, cnt):
        # strided view of a [128, INP] tile: columns (j0+jj)*7 + i
        v = ap_2d.rearrange("p (j i) -> p j i", i=P1)
        return v[:, j0 : j0 + cnt, i : i + 1].rearrange("p j i -> p (j i)")

    with tile.TileContext(nc) as tc:
        with (
            tc.tile_pool(name="const", bufs=1) as cpool,
            tc.tile_pool(name="zz", bufs=1) as zpool,
            tc.tile_pool(name="ysb", bufs=1) as ysbpool,
            tc.tile_pool(name="rk", bufs=1) as rkpool,
            tc.tile_pool(name="dram", bufs=1, space="DRAM") as dpool,
        ):
            # ---------------- constants ----------------
            ident = cpool.tile([128, 128], BF16, tag="ident", name="ident")
            masks.make_identity(nc, ident[:])
            ones_bf = cpool.tile([1, 128], BF16, tag="ones_bf", name="ones_bf")
            nc.vector.memset(ones_bf[:], 1.0)
            onesf = cpool.tile([1, 128], F32, tag="onesf", name="onesf")
            nc.vector.memset(onesf[:], 1.0)

            scal = cpool.tile([1, 4], F32, tag="scal", name="scal")
            for idx, ap in enumerate((a1, a2, b1, b2)):
                nc.sync.dma_start(scal[0:1, idx : idx + 1], ap[:])

            plb_f = cpool.tile([1, SEQ], F32, tag="plb_f", name="plb_f")
            nc.sync.dma_start(plb_f[:], plb[:])
            plb_sb = cpool.tile([1, SEQ], BF16, tag="plb", name="plb")
            nc.vector.tensor_copy(plb_sb[:], plb_f[:])

            densb = cpool.tile([128, 24], F32, tag="densb", name="densb")
            nc.vector.memset(densb[:], 0.0)
            shiftc = cpool.tile([128, 1], F32, tag="shiftc", name="shiftc")
            nc.vector.memset(shiftc[:], SHIFT)
            den_all = cpool.tile([128, 24], F32, tag="den_all", name="den_all")
            recip = cpool.tile([128, 24], F32, tag="recip", name="recip")

            cc_in = dpool.tile([128, 24], F32)
            cc_out = dpool.tile([128, 24], F32, addr_space="Shared")

            # z / y scan state fp32: c0|c1 packed [128,1024], c2 [32,512]
            zA = [
                zpool.tile([128, 2 * BL], F32, tag=f"zA{i}", name=f"zA{i}")
                for i in range(P1)
            ]
            zC = [
                zpool.tile([32, BL], F32, tag=f"zC{i}", name=f"zC{i}")
                for i in range(P1)
            ]
            # bf16 scan mirrors for the projection lhsT
            ysb = [
                [
                    ysbpool.tile([cnt, BL], BF16, tag=f"ysb{i}_{c}", name=f"ysb{i}_{c}")
                    for c, (j0, cnt) in enumerate(CH)
                ]
                for i in range(P1)
            ]
            # de-interleaved plw^T tiles [cnt, SEQ]
            rk = [
                [
                    rkpool.tile([cnt, SEQ], BF16, tag=f"rk{i}_{c}", name=f"rk{i}_{c}")
                    for c, (j0, cnt) in enumerate(CH)
                ]
                for i in range(P1)
            ]

            # ---------------- phase A: weights, x, logits, AR issue --------
            with (
                tc.tile_pool(name="wst", bufs=3) as wstage,
                tc.tile_pool(name="wbf", bufs=1) as wpool,
                tc.tile_pool(name="xst", bufs=2) as xstage,
                tc.tile_pool(name="xb", bufs=4) as xbpool,
                tc.tile_pool(name="xiT", bufs=1) as xtpool,
                tc.tile_pool(name="ee", bufs=1) as epool,
                tc.tile_pool(name="psT", bufs=2, space="PSUM") as psT,
                tc.tile_pool(name="psS", bufs=2, space="PSUM") as psS,
            ):
                # E (softmax numerator, bf16), packed like z
                EA = [
                    epool.tile([128, 2 * BL], BF16, tag=f"EA{i}", name=f"EA{i}")
                    for i in range(P1)
                ]
                EC = [
                    epool.tile([32, BL], BF16, tag=f"EC{i}", name=f"EC{i}")
                    for i in range(P1)
                ]
                # gate scalars -> all 128 partitions via PE
                pbc = psS.tile([128, 512], F32, tag="ps_st", name="ps_bc")
                nc.tensor.matmul(pbc[:, 0:4], onesf[:], scal[:], start=True, stop=True)
                bcast = cpool.tile([128, 4], F32, tag="bcast", name="bcast")
                nc.vector.tensor_copy(bcast[:], pbc[:, 0:4])

                # weights fp32 -> bf16 (contiguous casts)
                def load_w(src, nm):
                    tiles = []
                    for t, (m0, mc) in enumerate(CH):
                        wtf = wstage.tile([mc, N1], F32, tag="wtmp", name="wtmp")
                        nc.sync.dma_start(wtf[:], src[m0 : m0 + mc, :])
                        wt = wpool.tile([mc, N1], BF16, tag=f"{nm}{t}", name=f"{nm}{t}")
                        nc.vector.tensor_copy(wt[:], wtf[:])
                        tiles.append(wt)
                    return tiles

                wk_b = load_w(wk, "wkb")
                h1_b = load_w(h1, "h1b")
                wv_b = load_w(wv, "wvb")

                # x shard: fp32 load -> contiguous bf16 cast
                xbt = []
                for bt in range(4):
                    xt = xstage.tile([128, INP], F32, tag="xn", name="xn")
                    nc.sync.dma_start(xt[:], x[bt * 128 : (bt + 1) * 128, :])
                    xb = xbpool.tile([128, INP], BF16, tag="xnb", name="xnb")
                    nc.vector.tensor_copy(xb[:], xt[:])
                    xbt.append(xb)

                # h1T[l, j] = h1[j, l] and wkT[l, m] = wk[m, l]  (bf16)
                h1T, wkT = [], []
                for lt, (l0, lc) in enumerate(CH):
                    ps = psT.tile([128, 512], BF16, tag="tp", name="tp")
                    for jt, (j0, jc) in enumerate(CH):
                        nc.tensor.transpose(
                            ps[0:lc, j0 : j0 + jc],
                            h1_b[jt][:, l0 : l0 + lc],
                            ident[0:jc, 0:jc],
                        )
                    hT = wpool.tile([lc, N1], BF16, tag=f"h1T{lt}", name=f"h1T{lt}")
                    nc.vector.tensor_copy(hT[:], ps[0:lc, 0:N1])
                    h1T.append(hT)
                    ps2 = psT.tile([128, 512], BF16, tag="tp", name="tp")
                    for mt, (m0, mc) in enumerate(CH):
                        nc.tensor.transpose(
                            ps2[0:lc, m0 : m0 + mc],
                            wk_b[mt][:, l0 : l0 + lc],
                            ident[0:mc, 0:mc],
                        )
                    wTl = wpool.tile([lc, N1], BF16, tag=f"wkT{lt}", name=f"wkT{lt}")
                    nc.vector.tensor_copy(wTl[:], ps2[0:lc, 0:N1])
                    wkT.append(wTl)

                # W_hkT[m, j] = sum_l wk[m,l] h1[j,l]: lhsT=wkT, rhs=h1T (K=l)
                whkT = []
                for mt, (m0, mc) in enumerate(CH):
                    pw = psS.tile([128, 512], F32, tag="ps_st", name="ps_whk")
                    for lt, (l0, lc) in enumerate(CH):
                        nc.tensor.matmul(
                            pw[0:mc, 0:N1],
                            wkT[lt][:, m0 : m0 + mc],
                            h1T[lt][:],
                            start=(lt == 0),
                            stop=(lt == 2),
                        )
                    wTt = wpool.tile([mc, N1], BF16, tag=f"whkT{mt}", name=f"whkT{mt}")
                    nc.vector.tensor_copy(wTt[:], pw[0:mc, 0:N1])
                    whkT.append(wTt)

                # per slice: de-interleaving transposes, logits, exp (+accum)
                xiT = [[None] * 3 for _ in range(P1)]
                for i in range(P1):
                    for c, (j0, cnt) in enumerate(CH):
                        ps = psT.tile([128, 512], BF16, tag="tp", name="tp")
                        for bt in range(4):
                            nc.tensor.transpose(
                                ps[0:cnt, bt * 128 : (bt + 1) * 128],
                                deint(xbt[bt][:], i, j0, cnt),
                                ident[:],
                            )
                        xi = xtpool.tile(
                            [cnt, BL], BF16, tag=f"xiT{i}_{c}", name=f"xiT{i}_{c}"
                        )
                        if c % 2 == 0:
                            nc.vector.tensor_copy(xi[:], ps[0:cnt, :])
                        else:
                            nc.scalar.copy(xi[:], ps[0:cnt, :])
                        xiT[i][c] = xi

                    for jt, (j0, jc) in enumerate(CH):
                        pst = psS.tile([128, 512], F32, tag="ps_st", name="ps_st")
                        for lt, (l0, lc) in enumerate(CH):
                            nc.tensor.matmul(
                                pst[0:jc, :],
                                whkT[lt][:, j0 : j0 + jc],
                                xiT[i][lt][:],
                                start=(lt == 0),
                                stop=(lt == 2),
                            )
                        if jt < 2:
                            eout = EA[i][:, jt * BL : (jt + 1) * BL]
                        else:
                            eout = EC[i][:]
                        col = i * 3 + jt
                        nc.scalar.activation(
                            eout,
                            pst[0:jc, :],
                            AF.Exp,
                            bias=shiftc[0:jc, 0:1],
                            scale=SCALE,
                            accum_out=densb[0:jc, col : col + 1],
                        )

                # ---- AllReduce of exp-sums (hidden under vT + plw work) ----
                nc.gpsimd.dma_start(cc_in[:], densb[:])
                nc.gpsimd.collective_compute(
                    "AllReduce",
                    ALU.add,
                    replica_groups=[list(range(N_CORES))],
                    ins=[cc_in[:]],
                    outs=[cc_out[:]],
                )

                # ---- vT = (x_i @ wv)^T ; z = vT * E (normalized in scan) --
                for i in range(P1):
                    for ntc, (n0, ncnt) in enumerate(CH):
                        pv = psT.tile([128, 512], F32, tag="ps_vt", name="ps_vt")
                        for mt, (m0, mc) in enumerate(CH):
                            nc.tensor.matmul(
                                pv[0:ncnt, :],
                                wv_b[mt][:, n0 : n0 + ncnt],
                                xiT[i][mt][:],
                                start=(mt == 0),
                                stop=(mt == 2),
                            )
                        if ntc < 2:
                            zv = zA[i][:, ntc * BL : (ntc + 1) * BL]
                            ev = EA[i][:, ntc * BL : (ntc + 1) * BL]
                        else:
                            zv = zC[i][:]
                            ev = EC[i][:]
                        nc.vector.tensor_mul(zv, pv[0:ncnt, :], ev)

            # ---------------- phase B: plw pipeline (in the AR window) -----
            with (
                tc.tile_pool(name="plst", bufs=2) as plstage,
                tc.tile_pool(name="plb16", bufs=8) as pwbpool,
                tc.tile_pool(name="psT2", bufs=2, space="PSUM") as psT2,
            ):
                pwb = []
                for st in range(8):
                    pwt = plstage.tile([128, INP], F32, tag="plwn", name="plwn")
                    nc.sync.dma_start(pwt[:], plw[st * 128 : (st + 1) * 128, :])
                    pb = pwbpool.tile([128, INP], BF16, tag="plwb", name="plwb")
                    nc.vector.tensor_copy(pb[:], pwt[:])
                    pwb.append(pb)

                for i in range(P1):
                    for c, (j0, cnt) in enumerate(CH):
                        ps = psT2.tile([128, SEQ], BF16, tag="tp2", name="tp2")
                        for st in range(8):
                            nc.tensor.transpose(
                                ps[0:cnt, st * 128 : (st + 1) * 128],
                                deint(pwb[st][:], i, j0, cnt),
                                ident[:],
                            )
                        if (i * 3 + c) % 2 == 0:
                            nc.scalar.copy(rk[i][c][:], ps[0:cnt, :])
                        else:
                            nc.vector.tensor_copy(rk[i][c][:], ps[0:cnt, :])

            # ---------------- phase C: AR consume, scan + projection -------
            nc.gpsimd.dma_start(den_all[:], cc_out[:])
            nc.vector.reciprocal(recip[:], den_all[:])

            with (
                tc.tile_pool(name="tmp", bufs=1) as tmppool,
                tc.tile_pool(name="osb", bufs=4) as outpool,
                tc.tile_pool(name="psP", bufs=1, space="PSUM") as psP,
            ):
                ttA = tmppool.tile([128, 2 * BL], F32, tag="ttA", name="ttA")
                tsA = tmppool.tile([128, 2 * BL], F32, tag="tsA", name="tsA")
                gA = tmppool.tile([128, 2 * BL], F32, tag="gA", name="gA")
                tt2 = tmppool.tile([32, BL], F32, tag="tt2", name="tt2")
                ts2 = tmppool.tile([32, BL], F32, tag="ts2", name="ts2")
                g2 = tmppool.tile([32, BL], F32, tag="g2", name="g2")

                # pre-load proj_len_b into the 8 projection PSUM banks
                pps = {}
                for half in range(2):
                    for bc in range(4):
                        pp = psP.tile([128, 512], F32, tag=f"pj{half}{bc}", name=f"pj{half}{bc}")
                        nc.tensor.matmul(
                            pp[:],
                            ones_bf[:],
                            plb_sb[0:1, half * 512 : (half + 1) * 512],
                            start=True,
                            stop=False,
                        )
                        pps[(half, bc)] = pp

                def proj_slice(i, last):
                    for c, (j0, cnt) in enumerate(CH):
                        for half in range(2):
                            for bc in range(4):
                                nc.tensor.matmul(
                                    pps[(half, bc)][:],
                                    ysb[i][c][:, bc * 128 : (bc + 1) * 128],
                                    rk[i][c][:, half * 512 : (half + 1) * 512],
                                    start=False,
                                    stop=(last and c == 2 and half == 1 and bc == 3),
                                )

                for i in range(P1):
                    cols = [i * 3, i * 3 + 1, i * 3 + 2]
                    if i == 0:
                        # y_0 = z_0 * recip
                        for c in range(2):
                            nc.scalar.mul(
                                zA[0][:, c * BL : (c + 1) * BL],
                                zA[0][:, c * BL : (c + 1) * BL],
                                mul=recip[0:128, cols[c] : cols[c] + 1],
                            )
                        nc.scalar.mul(
                            zC[0][:], zC[0][:], mul=recip[0:32, cols[2] : cols[2] + 1]
                        )
                    else:
                        # g = tanh(a1*y + b1) * sigmoid(a2*y + b2)
                        nc.scalar.activation(
                            ttA[:], zA[i - 1][:], AF.Tanh,
                            bias=bcast[0:128, 2:3], scale=bcast[0:128, 0:1],
                        )
                        nc.scalar.activation(
                            tsA[:], zA[i - 1][:], AF.Sigmoid,
                            bias=bcast[0:128, 3:4], scale=bcast[0:128, 1:2],
                        )
                        nc.scalar.activation(
                            tt2[:], zC[i - 1][:], AF.Tanh,
                            bias=bcast[0:32, 2:3], scale=bcast[0:32, 0:1],
                        )
                        nc.scalar.activation(
                            ts2[:], zC[i - 1][:], AF.Sigmoid,
                            bias=bcast[0:32, 3:4], scale=bcast[0:32, 1:2],
                        )
                        nc.gpsimd.tensor_mul(gA[:], ttA[:], tsA[:])
                        nc.gpsimd.tensor_mul(g2[:], tt2[:], ts2[:])
                        # y_i = z_i * recip + g   (normalize folded in)
                        for c in range(2):
                            zv = zA[i][:, c * BL : (c + 1) * BL]
                            nc.vector.scalar_tensor_tensor(
                                zv, zv,
                                recip[0:128, cols[c] : cols[c] + 1],
                                gA[:, c * BL : (c + 1) * BL],
                                op0=ALU.mult, op1=ALU.add,
                            )
                        nc.vector.scalar_tensor_tensor(
                            zC[i][:], zC[i][:],
                            recip[0:32, cols[2] : cols[2] + 1],
                            g2[:],
                            op0=ALU.mult, op1=ALU.add,
                        )
                    # bf16 mirrors for the projection
                    nc.vector.tensor_copy(ysb[i][0][:], zA[i][:, 0:BL])
                    nc.vector.tensor_copy(ysb[i][1][:], zA[i][:, BL : 2 * BL])
                    nc.vector.tensor_copy(ysb[i][2][:], zC[i][:])
                    proj_slice(i, last=(i == P1 - 1))

                for half in range(2):
                    for bc in range(4):
                        ob = outpool.tile([128, 512], F32, tag="osb", name="osb")
                        nc.scalar.copy(ob[:], pps[(half, bc)][:])
                        nc.sync.dma_start(
                            out[bc * 128 : (bc + 1) * 128, half * 512 : (half + 1) * 512],
                            ob[:],
                        )

    nc.compile()
    return nc


_NC = None


def _get_nc():
    global _NC
    if _NC is None:
        _NC = build()
    return _NC


def run(inputs, trace=False):
    nc = _get_nc()
    rep_keys = [
        "w_k1",
        "w_v1",
        "h1",
        "alpha1",
        "alpha2",
        "beta1",
        "beta2",
        "proj_len_w",
        "proj_len_b",
    ]
    x = np.ascontiguousarray(inputs["x"], dtype=np.float32)
    rep = {k: np.ascontiguousarray(inputs[k], dtype=np.float32) for k in rep_keys}
    in_maps = [
        {"x": x[c * BL : (c + 1) * BL], **rep} for c in range(N_CORES)
    ]
    res = run_bass_kernel_spmd(
        nc, in_maps, core_ids=list(range(N_CORES)), trace=trace
    )
    full = np.concatenate([res.results[c]["out"] for c in range(N_CORES)], axis=0)
    return full, res


def kernel(**inputs):
    full, _ = run(inputs, trace=False)
    return full


# revision 9
# speedup vs baseline: 1.1453x; 1.0742x over previous
"""Trainium2 Bass kernel for nn_InternalMAFE_59270548684863.

Key facts (hardcoded from the problem):
  - Output depends ONLY on branch 1 (p=7, n=288) of the reference; the
    n2=1008 branch feeds a dead projection and is never computed.
  - out = o1 @ proj_len_w.T + proj_len_b,  o1 = branch(x, 7, h1, w_k1, w_v1, ...)
  - Softmax normalizes over the batch axis, so we batch-shard (512 rows/core)
    and AllReduce the per-(slice, feature) exp-sums. Constant-shift softmax
    (exp(s*scale - 50)) avoids a cross-core max pass.
  - s = h1 @ (x_i w_k)^T is fused as W_hk = h1 @ w_k^T (one 288^3 product).
  - All matmuls run in bf16; PSUM accumulation stays fp32.

v3 schedule:
  - fp32->bf16 casts are contiguous (fast DVE mode); the feature
    de-interleave (stride 7) happens inside the PE transposes via strided
    stationary-operand views.
  - The exp-sum AllReduce is split in two (slices 0-3 / 4-6) and issued as
    early as possible; waits hide under vT matmuls + the plw pipeline.
  - Scan state is bf16 and doubles as the projection lhsT (no mirror casts).
    Softmax normalization folds into the scan via scalar_tensor_tensor.
  - The big-gate product and normalize+add run on DVE in bf16 (2x mode);
    the small c2 chunk runs on the otherwise-idle GPSIMD engine.
  - Projection accumulates per-slice into 8 PSUM banks while the scan runs,
    with proj_len_b pre-loaded via a ones-outer-product matmul.
"""

import math

import numpy as np

import concourse.bacc as bacc
import concourse.masks as masks
import concourse.mybir as mybir
import concourse.tile as tile
from concourse.bass_utils import run_bass_kernel_spmd

N_CORES = 8
B = 4096
BL = B // N_CORES  # 512 rows per core
INP = 2016
P1 = 7
N1 = 288
SEQ = 1024
SCALE = 1.0 / math.sqrt(N1)
SHIFT = -50.0
F32 = mybir.dt.float32
BF16 = mybir.dt.bfloat16
CH = [(0, 128), (128, 128), (256, 32)]
AF = mybir.ActivationFunctionType
ALU = mybir.AluOpType


def build():
    nc = bacc.Bacc(
        "TRN2", target_bir_lowering=False, debug=False, num_devices=N_CORES
    )
    x = nc.dram_tensor("x", [BL, INP], F32, kind="ExternalInput").ap()
    wk = nc.dram_tensor("w_k1", [N1, N1], F32, kind="ExternalInput").ap()
    wv = nc.dram_tensor("w_v1", [N1, N1], F32, kind="ExternalInput").ap()
    h1 = nc.dram_tensor("h1", [N1, N1], F32, kind="ExternalInput").ap()
    a1 = nc.dram_tensor("alpha1", [1], F32, kind="ExternalInput").ap()
    a2 = nc.dram_tensor("alpha2", [1], F32, kind="ExternalInput").ap()
    b1 = nc.dram_tensor("beta1", [1], F32, kind="ExternalInput").ap()
    b2 = nc.dram_tensor("beta2", [1], F32, kind="ExternalInput").ap()
    plw = nc.dram_tensor("proj_len_w", [SEQ, INP], F32, kind="ExternalInput").ap()
    plb = nc.dram_tensor("proj_len_b", [SEQ], F32, kind="ExternalInput").ap()
    out = nc.dram_tensor("out", [BL, SEQ], F32, kind="ExternalOutput").ap()

    def deint(ap_2d, i, j0, cnt):
        # strided view of a [128, INP] tile: columns (j0+jj)*7 + i
        v = ap_2d.rearrange("p (j i) -> p j i", i=P1)
        return v[:, j0 : j0 + cnt, i : i + 1].rearrange("p j i -> p (j i)")

    with tile.TileContext(nc) as tc:
        with (
            tc.tile_pool(name="const", bufs=1) as cpool,
            tc.tile_pool(name="zz", bufs=1) as zpool,
            tc.tile_pool(name="rk", bufs=1) as rkpool,
            tc.tile_pool(name="dram", bufs=1, space="DRAM") as dpool,
        ):
            # ---------------- constants ----------------
            ident = cpool.tile([128, 128], BF16, tag="ident", name="ident")
            masks.make_identity(nc, ident[:])
            ones_bf = cpool.tile([1, 128], BF16, tag="ones_bf", name="ones_bf")
            nc.vector.memset(ones_bf[:], 1.0)
            onesf = cpool.tile([1, 128], F32, tag="onesf", name="onesf")
            nc.vector.memset(onesf[:], 1.0)

            scal = cpool.tile([1, 4], F32, tag="scal", name="scal")
            for idx, ap in enumerate((a1, a2, b1, b2)):
                nc.sync.dma_start(scal[0:1, idx : idx + 1], ap[:])

            plb_f = cpool.tile([1, SEQ], F32, tag="plb_f", name="plb_f")
            nc.sync.dma_start(plb_f[:], plb[:])
            plb_sb = cpool.tile([1, SEQ], BF16, tag="plb", name="plb")
            nc.vector.tensor_copy(plb_sb[:], plb_f[:])

            densb = cpool.tile([128, 24], F32, tag="densb", name="densb")
            nc.vector.memset(densb[:], 0.0)
            shiftc = cpool.tile([128, 1], F32, tag="shiftc", name="shiftc")
            nc.vector.memset(shiftc[:], SHIFT)
            den_all = cpool.tile([128, 24], F32, tag="den_all", name="den_all")
            recip = cpool.tile([128, 24], F32, tag="recip", name="recip")

            cc_in = [
                dpool.tile([128, 12], F32, tag=f"cc_in{h}", name=f"cc_in{h}")
                for h in range(2)
            ]
            cc_out = [
                dpool.tile(
                    [128, 12], F32, addr_space="Shared",
                    tag=f"cc_out{h}", name=f"cc_out{h}",
                )
                for h in range(2)
            ]

            # scan state, bf16 (doubles as projection lhsT):
            # c0|c1 packed [128, 1024], c2 [32, 512]
            zA = [
                zpool.tile([128, 2 * BL], BF16, tag=f"zA{i}", name=f"zA{i}")
                for i in range(P1)
            ]
            zC = [
                zpool.tile([32, BL], BF16, tag=f"zC{i}", name=f"zC{i}")
                for i in range(P1)
            ]
            # de-interleaved plw^T tiles [cnt, SEQ]
            rk = [
                [
                    rkpool.tile([cnt, SEQ], BF16, tag=f"rk{i}_{c}", name=f"rk{i}_{c}")
                    for c, (j0, cnt) in enumerate(CH)
                ]
                for i in range(P1)
            ]

            # ---------------- phase A ----------------
            with (
                tc.tile_pool(name="wst", bufs=3) as wstage,
                tc.tile_pool(name="wbf", bufs=1) as wpool,
                tc.tile_pool(name="stg", bufs=2) as stage,
                tc.tile_pool(name="xb", bufs=4) as xbpool,
                tc.tile_pool(name="pb16", bufs=4) as pwbpool,
                tc.tile_pool(name="xiT", bufs=1) as xtpool,
                tc.tile_pool(name="ee", bufs=1) as epool,
                tc.tile_pool(name="psT", bufs=3, space="PSUM") as psT,
                tc.tile_pool(name="psS", bufs=2, space="PSUM") as psS,
                tc.tile_pool(name="psT2", bufs=2, space="PSUM") as psT2,
            ):
                EA = [
                    epool.tile([128, 2 * BL], BF16, tag=f"EA{i}", name=f"EA{i}")
                    for i in range(P1)
                ]
                EC = [
                    epool.tile([32, BL], BF16, tag=f"EC{i}", name=f"EC{i}")
                    for i in range(P1)
                ]

                # gate scalars -> all 128 partitions via PE
                pbc = psS.tile([128, 512], F32, tag="ps_st", name="ps_bc")
                nc.tensor.matmul(pbc[:, 0:4], onesf[:], scal[:], start=True, stop=True)
                bcast = cpool.tile([128, 4], F32, tag="bcast", name="bcast")
                nc.vector.tensor_copy(bcast[:], pbc[:, 0:4])

                # weights fp32 -> bf16 (contiguous casts)
                def load_w(src, nm):
                    tiles = []
                    for t, (m0, mc) in enumerate(CH):
                        wtf = wstage.tile([mc, N1], F32, tag="wtmp", name="wtmp")
                        nc.sync.dma_start(wtf[:], src[m0 : m0 + mc, :])
                        wt = wpool.tile([mc, N1], BF16, tag=f"{nm}{t}", name=f"{nm}{t}")
                        nc.vector.tensor_copy(wt[:], wtf[:])
                        tiles.append(wt)
                    return tiles

                wk_b = load_w(wk, "wkb")
                h1_b = load_w(h1, "h1b")
                wv_b = load_w(wv, "wvb")

                # x shard: fp32 load -> contiguous bf16 cast
                xbt = []
                for bt in range(4):
                    xt = stage.tile([128, INP], F32, tag="stg", name="stg")
                    nc.sync.dma_start(xt[:], x[bt * 128 : (bt + 1) * 128, :])
                    xb = xbpool.tile([128, INP], BF16, tag="xnb", name="xnb")
                    nc.vector.tensor_copy(xb[:], xt[:])
                    xbt.append(xb)

                # plw: fp32 load -> contiguous bf16 cast (DMAs queue behind x)
                pwb = []
                for st in range(8):
                    pwt = stage.tile([128, INP], F32, tag="stg", name="stg")
                    nc.sync.dma_start(pwt[:], plw[st * 128 : (st + 1) * 128, :])
                    pb = pwbpool.tile([128, INP], BF16, tag="plwb", name="plwb")
                    nc.vector.tensor_copy(pb[:], pwt[:])
                    pwb.append(pb)

                # h1T[l, j] = h1[j, l] and wkT[l, m] = wk[m, l]  (bf16)
                h1T, wkT = [], []
                for lt, (l0, lc) in enumerate(CH):
                    ps = psT.tile([128, 512], BF16, tag="tp", name="tp")
                    for jt, (j0, jc) in enumerate(CH):
                        nc.tensor.transpose(
                            ps[0:lc, j0 : j0 + jc],
                            h1_b[jt][:, l0 : l0 + lc],
                            ident[0:jc, 0:jc],
                        )
                    hT = wpool.tile([lc, N1], BF16, tag=f"h1T{lt}", name=f"h1T{lt}")
                    nc.vector.tensor_copy(hT[:], ps[0:lc, 0:N1])
                    h1T.append(hT)
                    ps2 = psT.tile([128, 512], BF16, tag="tp", name="tp")
                    for mt, (m0, mc) in enumerate(CH):
                        nc.tensor.transpose(
                            ps2[0:lc, m0 : m0 + mc],
                            wk_b[mt][:, l0 : l0 + lc],
                            ident[0:mc, 0:mc],
                        )
                    wTl = wpool.tile([lc, N1], BF16, tag=f"wkT{lt}", name=f"wkT{lt}")
                    nc.vector.tensor_copy(wTl[:], ps2[0:lc, 0:N1])
                    wkT.append(wTl)

                # W_hkT[m, j] = sum_l wk[m,l] h1[j,l]
                whkT = []
                for mt, (m0, mc) in enumerate(CH):
                    pw = psS.tile([128, 512], F32, tag="ps_st", name="ps_whk")
                    for lt, (l0, lc) in enumerate(CH):
                        nc.tensor.matmul(
                            pw[0:mc, 0:N1],
                            wkT[lt][:, m0 : m0 + mc],
                            h1T[lt][:],
                            start=(lt == 0),
                            stop=(lt == 2),
                        )
                    wTt = wpool.tile([mc, N1], BF16, tag=f"whkT{mt}", name=f"whkT{mt}")
                    nc.vector.tensor_copy(wTt[:], pw[0:mc, 0:N1])
                    whkT.append(wTt)

                # software-pipelined: transposes of slice i+1 are emitted
                # before the logits of slice i so PE never waits on copybacks
                xiT = [[None] * 3 for _ in range(P1)]

                def emit_transposes(i):
                    for c, (j0, cnt) in enumerate(CH):
                        ps = psT.tile([128, 512], BF16, tag="tp", name="tp")
                        for bt in range(4):
                            nc.tensor.transpose(
                                ps[0:cnt, bt * 128 : (bt + 1) * 128],
                                deint(xbt[bt][:], i, j0, cnt),
                                ident[:],
                            )
                        xi = xtpool.tile(
                            [cnt, BL], BF16, tag=f"xiT{i}_{c}", name=f"xiT{i}_{c}"
                        )
                        if c % 2 == 0:
                            nc.vector.tensor_copy(xi[:], ps[0:cnt, :])
                        else:
                            nc.scalar.copy(xi[:], ps[0:cnt, :])
                        xiT[i][c] = xi

                def emit_logits(i):
                    for jt, (j0, jc) in enumerate(CH):
                        pst = psS.tile([128, 512], F32, tag="ps_st", name="ps_st")
                        for lt, (l0, lc) in enumerate(CH):
                            nc.tensor.matmul(
                                pst[0:jc, :],
                                whkT[lt][:, j0 : j0 + jc],
                                xiT[i][lt][:],
                                start=(lt == 0),
                                stop=(lt == 2),
                            )
                        if jt < 2:
                            eout = EA[i][:, jt * BL : (jt + 1) * BL]
                        else:
                            eout = EC[i][:]
                        col = i * 3 + jt
                        nc.scalar.activation(
                            eout,
                            pst[0:jc, :],
                            AF.Exp,
                            bias=shiftc[0:jc, 0:1],
                            scale=SCALE,
                            accum_out=densb[0:jc, col : col + 1],
                        )

                emit_transposes(0)
                for i in range(P1):
                    if i + 1 < P1:
                        emit_transposes(i + 1)
                    emit_logits(i)
                    # split AllReduce: slices 0-3 then 4-6
                    if i == 3:
                        nc.gpsimd.dma_start(cc_in[0][:], densb[:, 0:12])
                        nc.gpsimd.collective_compute(
                            "AllReduce",
                            ALU.add,
                            replica_groups=[list(range(N_CORES))],
                            ins=[cc_in[0][:]],
                            outs=[cc_out[0][:]],
                        )
                nc.gpsimd.dma_start(cc_in[1][:], densb[:, 12:24])
                nc.gpsimd.collective_compute(
                    "AllReduce",
                    ALU.add,
                    replica_groups=[list(range(N_CORES))],
                    ins=[cc_in[1][:]],
                    outs=[cc_out[1][:]],
                )

                # vT = (x_i @ wv)^T ; z = vT * E  (normalized in the scan)
                for i in range(P1):
                    for ntc, (n0, ncnt) in enumerate(CH):
                        pv = psS.tile([128, 512], F32, tag="ps_st", name="ps_vt")
                        for mt, (m0, mc) in enumerate(CH):
                            nc.tensor.matmul(
                                pv[0:ncnt, :],
                                wv_b[mt][:, n0 : n0 + ncnt],
                                xiT[i][mt][:],
                                start=(mt == 0),
                                stop=(mt == 2),
                            )
                        if ntc < 2:
                            zv = zA[i][:, ntc * BL : (ntc + 1) * BL]
                            ev = EA[i][:, ntc * BL : (ntc + 1) * BL]
                        else:
                            zv = zC[i][:]
                            ev = EC[i][:]
                        nc.vector.tensor_mul(zv, pv[0:ncnt, :], ev)

                # plw de-interleave (fills the AllReduce window), per seq-half
                for sh in range(2):
                    for i in range(P1):
                        for c, (j0, cnt) in enumerate(CH):
                            ps = psT2.tile([128, 512], BF16, tag="tp2", name="tp2")
                            for st in range(4):
                                nc.tensor.transpose(
                                    ps[0:cnt, st * 128 : (st + 1) * 128],
                                    deint(pwb[sh * 4 + st][:], i, j0, cnt),
                                    ident[:],
                                )
                            dst = rk[i][c][:, sh * 512 : (sh + 1) * 512]
                            if (i * 3 + c) % 2 == 0:
                                nc.scalar.copy(dst, ps[0:cnt, :])
                            else:
                                nc.vector.tensor_copy(dst, ps[0:cnt, :])

            # ---------------- AR consume ----------------
            nc.gpsimd.dma_start(den_all[:, 0:12], cc_out[0][:])
            nc.vector.reciprocal(recip[:, 0:12], den_all[:, 0:12])
            nc.gpsimd.dma_start(den_all[:, 12:24], cc_out[1][:])
            nc.vector.reciprocal(recip[:, 12:24], den_all[:, 12:24])

            # ---------------- scan + projection ----------------
            with (
                tc.tile_pool(name="tmp", bufs=1) as tmppool,
                tc.tile_pool(name="osb", bufs=2) as outpool,
                tc.tile_pool(name="psP", bufs=1, space="PSUM") as psP,
            ):
                ttA = tmppool.tile([128, 2 * BL], BF16, tag="ttA", name="ttA")
                tsA = tmppool.tile([128, 2 * BL], BF16, tag="tsA", name="tsA")
                gA = tmppool.tile([128, 2 * BL], BF16, tag="gA", name="gA")
                tt2 = tmppool.tile([32, BL], BF16, tag="tt2", name="tt2")
                ts2 = tmppool.tile([32, BL], BF16, tag="ts2", name="ts2")
                g2 = tmppool.tile([32, BL], BF16, tag="g2", name="g2")

                # pre-load proj_len_b into the 8 projection PSUM banks
                pps = {}
                for half in range(2):
                    for bc in range(4):
                        pp = psP.tile(
                            [128, 512], F32, tag=f"pj{half}{bc}", name=f"pj{half}{bc}"
                        )
                        nc.tensor.matmul(
                            pp[:],
                            ones_bf[:],
                            plb_sb[0:1, half * 512 : (half + 1) * 512],
                            start=True,
                            stop=False,
                        )
                        pps[(half, bc)] = pp

                def proj_slice(i, last):
                    for c, (j0, cnt) in enumerate(CH):
                        if c < 2:
                            lsrc = zA[i]
                            off = c * BL
                        else:
                            lsrc = zC[i]
                            off = 0
                        for half in range(2):
                            for bc in range(4):
                                nc.tensor.matmul(
                                    pps[(half, bc)][:],
                                    lsrc[:, off + bc * 128 : off + (bc + 1) * 128],
                                    rk[i][c][:, half * 512 : (half + 1) * 512],
                                    start=False,
                                    stop=(last and c == 2 and half == 1 and bc == 3),
                                )

                for i in range(P1):
                    cols = [i * 3, i * 3 + 1, i * 3 + 2]
                    if i == 0:
                        # y_0 = z_0 * recip
                        for c in range(2):
                            nc.scalar.mul(
                                zA[0][:, c * BL : (c + 1) * BL],
                                zA[0][:, c * BL : (c + 1) * BL],
                                mul=recip[0:128, cols[c] : cols[c] + 1],
                            )
                        nc.scalar.mul(
                            zC[0][:], zC[0][:], mul=recip[0:32, cols[2] : cols[2] + 1]
                        )
                    else:
                        # g = tanh(a1*y + b1) * sigmoid(a2*y + b2)
                        nc.scalar.activation(
                            ttA[:], zA[i - 1][:], AF.Tanh,
                            bias=bcast[0:128, 2:3], scale=bcast[0:128, 0:1],
                        )
                        nc.scalar.activation(
                            tsA[:], zA[i - 1][:], AF.Sigmoid,
                            bias=bcast[0:128, 3:4], scale=bcast[0:128, 1:2],
                        )
                        nc.scalar.activation(
                            tt2[:], zC[i - 1][:], AF.Tanh,
                            bias=bcast[0:32, 2:3], scale=bcast[0:32, 0:1],
                        )
                        nc.scalar.activation(
                            ts2[:], zC[i - 1][:], AF.Sigmoid,
                            bias=bcast[0:32, 3:4], scale=bcast[0:32, 1:2],
                        )
                        nc.vector.tensor_mul(gA[:], ttA[:], tsA[:])
                        nc.gpsimd.tensor_mul(g2[:], tt2[:], ts2[:])
                        # y_i = z_i * recip + g   (normalize folded in)
                        for c in range(2):
                            zv = zA[i][:, c * BL : (c + 1) * BL]
                            nc.vector.scalar_tensor_tensor(
                                zv, zv,
                                recip[0:128, cols[c] : cols[c] + 1],
                                gA[:, c * BL : (c + 1) * BL],
                                op0=ALU.mult, op1=ALU.add,
                            )
                        nc.vector.scalar_tensor_tensor(
                            zC[i][:], zC[i][:],
                            recip[0:32, cols[2] : cols[2] + 1],
                            g2[:],
                            op0=ALU.mult, op1=ALU.add,
                        )
                    proj_slice(i, last=(i == P1 - 1))

                for half in range(2):
                    for bc in range(4):
                        ob = outpool.tile([128, 512], F32, tag="osb", name="osb")
                        nc.scalar.copy(ob[:], pps[(half, bc)][:])
                        nc.sync.dma_start(
                            out[bc * 128 : (bc + 1) * 128, half * 512 : (half + 1) * 512],
                            ob[:],
                        )

    nc.compile()
    return nc


_NC = None


def _get_nc():
    global _NC
    if _NC is None:
        _NC = build()
    return _NC


def run(inputs, trace=False):
    nc = _get_nc()
    rep_keys = [
        "w_k1",
        "w_v1",
        "h1",
        "alpha1",
        "alpha2",
        "beta1",
        "beta2",
        "proj_len_w",
        "proj_len_b",
    ]
    x = np.ascontiguousarray(inputs["x"], dtype=np.float32)
    rep = {k: np.ascontiguousarray(inputs[k], dtype=np.float32) for k in rep_keys}
    in_maps = [
        {"x": x[c * BL : (c + 1) * BL], **rep} for c in range(N_CORES)
    ]
    res = run_bass_kernel_spmd(
        nc, in_maps, core_ids=list(range(N_CORES)), trace=trace
    )
    full = np.concatenate([res.results[c]["out"] for c in range(N_CORES)], axis=0)
    return full, res


def kernel(**inputs):
    full, _ = run(inputs, trace=False)
    return full


# revision 26
# speedup vs baseline: 1.2339x; 1.0774x over previous
"""Trainium2 Bass kernel for nn_InternalMAFE_59270548684863.

Key facts (hardcoded from the problem):
  - Output depends ONLY on branch 1 (p=7, n=288) of the reference; the
    n2=1008 branch feeds a dead projection and is never computed.
  - out = o1 @ proj_len_w.T + proj_len_b,  o1 = branch(x, 7, h1, w_k1, w_v1, ...)
  - Softmax normalizes over the batch axis, so we batch-shard (512 rows/core)
    and AllReduce the per-(slice, feature) exp-sums. Constant-shift softmax
    (exp(s*scale - 50)) avoids a cross-core max pass.
  - s = h1 @ (x_i w_k)^T is fused as W_hk = h1 @ w_k^T (one 288^3 product).
  - All matmuls run in bf16; PSUM accumulation stays fp32.

v4 schedule:
  - Contiguous fp32->bf16 casts; the feature de-interleave (stride 7) happens
    inside the PE transposes via strided stationary-operand views.
  - Per-slice pipeline: transposes(i+1) | logits(i)+exp(i) | vT(i)+z-mul(i),
    keeping PE dense.  Split AllReduce (slices 0-3 after slice 3, 4-6 after
    slice 6).  The plw pipeline has all-resident bf16 buffers and its casts
    are emitted mid-FIFO so nothing stalls.
  - Scan state is bf16 and doubles as the projection lhsT; normalization is
    folded into the scan via scalar_tensor_tensor.  The projection
    accumulates per-slice into 8 PSUM banks while the scan runs.
"""

import math

import numpy as np

import concourse.bacc as bacc
import concourse.masks as masks
import concourse.mybir as mybir
import concourse.tile as tile
from concourse.bass_utils import run_bass_kernel_spmd

N_CORES = 8
B = 4096
BL = B // N_CORES  # 512 rows per core
INP = 2016
P1 = 7
N1 = 288
SEQ = 1024
SCALE = 1.0 / math.sqrt(N1)
SHIFT = -50.0
F32 = mybir.dt.float32
BF16 = mybir.dt.bfloat16
CH = [(0, 128), (128, 128), (256, 32)]
AF = mybir.ActivationFunctionType
ALU = mybir.AluOpType


def build():
    nc = bacc.Bacc(
        "TRN2", target_bir_lowering=False, debug=False, num_devices=N_CORES
    )
    x = nc.dram_tensor("x", [BL, INP], F32, kind="ExternalInput").ap()
    wk = nc.dram_tensor("w_k1", [N1, N1], F32, kind="ExternalInput").ap()
    wv = nc.dram_tensor("w_v1", [N1, N1], F32, kind="ExternalInput").ap()
    h1 = nc.dram_tensor("h1", [N1, N1], F32, kind="ExternalInput").ap()
    a1 = nc.dram_tensor("alpha1", [1], F32, kind="ExternalInput").ap()
    a2 = nc.dram_tensor("alpha2", [1], F32, kind="ExternalInput").ap()
    b1 = nc.dram_tensor("beta1", [1], F32, kind="ExternalInput").ap()
    b2 = nc.dram_tensor("beta2", [1], F32, kind="ExternalInput").ap()
    plw = nc.dram_tensor("proj_len_w", [SEQ, INP], F32, kind="ExternalInput").ap()
    plb = nc.dram_tensor("proj_len_b", [SEQ], F32, kind="ExternalInput").ap()
    out = nc.dram_tensor("out", [BL, SEQ], F32, kind="ExternalOutput").ap()

    def deint(ap_2d, i, j0, cnt):
        # strided view of a [128, INP] tile: columns (j0+jj)*7 + i
        v = ap_2d.rearrange("p (j i) -> p j i", i=P1)
        return v[:, j0 : j0 + cnt, i : i + 1].rearrange("p j i -> p (j i)")

    with tile.TileContext(nc) as tc:
        with (
            tc.tile_pool(name="const", bufs=1) as cpool,
            tc.tile_pool(name="zz", bufs=1) as zpool,
            tc.tile_pool(name="rk", bufs=1) as rkpool,
            tc.tile_pool(name="dram", bufs=1, space="DRAM") as dpool,
        ):
            # ---------------- constants ----------------
            ident = cpool.tile([128, 128], BF16, tag="ident", name="ident")
            masks.make_identity(nc, ident[:])
            ones_bf = cpool.tile([1, 128], BF16, tag="ones_bf", name="ones_bf")
            nc.vector.memset(ones_bf[:], 1.0)
            onesf = cpool.tile([1, 128], F32, tag="onesf", name="onesf")
            nc.vector.memset(onesf[:], 1.0)

            scal = cpool.tile([1, 4], F32, tag="scal", name="scal")
            for idx, ap in enumerate((a1, a2, b1, b2)):
                nc.sync.dma_start(scal[0:1, idx : idx + 1], ap[:])

            plb_f = cpool.tile([1, SEQ], F32, tag="plb_f", name="plb_f")
            nc.sync.dma_start(plb_f[:], plb[:])
            plb_sb = cpool.tile([1, SEQ], BF16, tag="plb", name="plb")
            nc.vector.tensor_copy(plb_sb[:], plb_f[:])

            densb = cpool.tile([128, 24], F32, tag="densb", name="densb")
            nc.vector.memset(densb[:], 0.0)
            shiftc = cpool.tile([128, 1], F32, tag="shiftc", name="shiftc")
            nc.vector.memset(shiftc[:], SHIFT)
            den_all = cpool.tile([128, 24], F32, tag="den_all", name="den_all")
            recip = cpool.tile([128, 24], F32, tag="recip", name="recip")

            cc_in = [
                dpool.tile([128, 12], F32, tag=f"cc_in{h}", name=f"cc_in{h}")
                for h in range(2)
            ]
            cc_out = [
                dpool.tile(
                    [128, 12], F32, addr_space="Shared",
                    tag=f"cc_out{h}", name=f"cc_out{h}",
                )
                for h in range(2)
            ]

            # scan state, bf16 (doubles as projection lhsT):
            # c0|c1 packed [128, 1024]; c2 packed 3-slices-per-tile [96, 512]
            zA = [
                zpool.tile([128, 2 * BL], BF16, tag=f"zA{i}", name=f"zA{i}")
                for i in range(P1)
            ]
            zC = [
                zpool.tile([32, BL], BF16, tag=f"zC{i}", name=f"zC{i}")
                for i in range(P1)
            ]
            # de-interleaved plw^T tiles
            rk = [
                [
                    rkpool.tile([128, SEQ], BF16, tag=f"rk{i}_{c}", name=f"rk{i}_{c}")
                    for c in range(2)
                ]
                for i in range(P1)
            ]
            rk2 = [
                rkpool.tile([32, SEQ], BF16, tag=f"rkc2_{i}", name=f"rkc2_{i}")
                for i in range(P1)
            ]

            # ---------------- phase A ----------------
            with (
                tc.tile_pool(name="wst", bufs=3) as wstage,
                tc.tile_pool(name="wbf", bufs=1) as wpool,
                tc.tile_pool(name="stg", bufs=2) as stage,
                tc.tile_pool(name="xb", bufs=4) as xbpool,
                tc.tile_pool(name="pb16", bufs=8) as pwbpool,
                tc.tile_pool(name="xiT", bufs=3) as xtpool,
                tc.tile_pool(name="ee", bufs=3) as epool,
                tc.tile_pool(name="psT", bufs=2, space="PSUM") as psT,
                tc.tile_pool(name="psS", bufs=2, space="PSUM") as psS,
                tc.tile_pool(name="psV", bufs=2, space="PSUM") as psV,
                tc.tile_pool(name="psT2", bufs=2, space="PSUM") as psT2,
            ):
                EA = [
                    epool.tile([128, 2 * BL], BF16, tag="EA", name=f"EA{i}")
                    for i in range(P1)
                ]
                EC = [
                    epool.tile([32, BL], BF16, tag="EC", name=f"EC{i}")
                    for i in range(P1)
                ]

                # PE warm-up: dependency-free transposes so the HAM clock
                # gate opens during the initial DMA window (results unused)
                for w in range(90):
                    psj = psS.tile([128, 512], BF16, tag="ps_st", name="ps_jk")
                    nc.tensor.transpose(psj[:, 0:128], ident[:], ident[:])

                # gate scalars -> all 128 partitions via PE
                pbc = psS.tile([128, 512], F32, tag="ps_st", name="ps_bc")
                nc.tensor.matmul(pbc[:, 0:4], onesf[:], scal[:], start=True, stop=True)
                bcast = cpool.tile([128, 4], F32, tag="bcast", name="bcast")
                nc.vector.tensor_copy(bcast[:], pbc[:, 0:4])

                def load_w(src, nm):
                    tiles = []
                    for t, (m0, mc) in enumerate(CH):
                        wtf = wstage.tile([mc, N1], F32, tag="wtmp", name="wtmp")
                        nc.sync.dma_start(wtf[:], src[m0 : m0 + mc, :])
                        wt = wpool.tile(
                            [mc, N1], BF16, tag=f"{nm}{t}", name=f"{nm}{t}"
                        )
                        nc.vector.tensor_copy(wt[:], wtf[:])
                        tiles.append(wt)
                    return tiles

                wk_b = load_w(wk, "wkb")
                h1_b = load_w(h1, "h1b")
                wv_b = load_w(wv, "wvb")

                # x shard: fp32 load -> contiguous bf16 cast
                xbt = []
                for bt in range(4):
                    xt = stage.tile([128, INP], F32, tag="stg", name="stg")
                    nc.sync.dma_start(xt[:], x[bt * 128 : (bt + 1) * 128, :])
                    xb = xbpool.tile([128, INP], BF16, tag="xnb", name="xnb")
                    nc.vector.tensor_copy(xb[:], xt[:])
                    xbt.append(xb)

                # plw DMAs (queue behind x; casts are emitted later mid-FIFO)
                pwstage = []
                for st in range(8):
                    pwt = stage.tile([128, INP], F32, tag="stg", name="stg")
                    nc.sync.dma_start(pwt[:], plw[st * 128 : (st + 1) * 128, :])
                    pwstage.append(pwt)
                pwb = [None] * 8

                def emit_plw_cast(st):
                    pb = pwbpool.tile([128, INP], BF16, tag="plwb", name="plwb")
                    nc.vector.tensor_copy(pb[:], pwstage[st][:])
                    pwb[st] = pb

                # h1T[l, j] = h1[j, l] and wkT[l, m] = wk[m, l]  (bf16)
                h1T, wkT = [], []
                for lt, (l0, lc) in enumerate(CH):
                    ps = psT.tile([128, 512], BF16, tag="tp", name="tp")
                    for jt, (j0, jc) in enumerate(CH):
                        nc.tensor.transpose(
                            ps[0:lc, j0 : j0 + jc],
                            h1_b[jt][0:jc, l0 : l0 + lc],
                            ident[0:jc, 0:jc],
                        )
                    hT = wpool.tile([lc, N1], BF16, tag=f"h1T{lt}", name=f"h1T{lt}")
                    nc.vector.tensor_copy(hT[:], ps[0:lc, 0:N1])
                    h1T.append(hT)
                    ps2 = psT.tile([128, 512], BF16, tag="tp", name="tp")
                    for mt, (m0, mc) in enumerate(CH):
                        nc.tensor.transpose(
                            ps2[0:lc, m0 : m0 + mc],
                            wk_b[mt][0:mc, l0 : l0 + lc],
                            ident[0:mc, 0:mc],
                        )
                    wTl = wpool.tile([lc, N1], BF16, tag=f"wkT{lt}", name=f"wkT{lt}")
                    nc.vector.tensor_copy(wTl[:], ps2[0:lc, 0:N1])
                    wkT.append(wTl)

                # W_hkT[m, j] = sum_l wk[m,l] h1[j,l]
                whkT = []
                for mt, (m0, mc) in enumerate(CH):
                    pw = psS.tile([128, 512], F32, tag="ps_st", name="ps_whk")
                    for lt, (l0, lc) in enumerate(CH):
                        nc.tensor.matmul(
                            pw[0:mc, 0:N1],
                            wkT[lt][:, m0 : m0 + mc],
                            h1T[lt][:],
                            start=(lt == 0),
                            stop=(lt == 2),
                        )
                    wTt = wpool.tile(
                        [mc, N1], BF16, tag=f"whkT{mt}", name=f"whkT{mt}"
                    )
                    nc.vector.tensor_copy(wTt[:], pw[0:mc, 0:N1])
                    whkT.append(wTt)

                # per-slice pipeline ---------------------------------------
                xiT = [[None] * 3 for _ in range(P1)]

                def emit_transposes(i):
                    for c in range(2):
                        j0 = c * 128
                        ps = psT.tile([128, 512], BF16, tag="tp", name="tp")
                        for bt in range(4):
                            nc.tensor.transpose(
                                ps[:, bt * 128 : (bt + 1) * 128],
                                deint(xbt[bt][:], i, j0, 128),
                                ident[:],
                            )
                        xi = xtpool.tile(
                            [128, BL], BF16, tag=f"xc{c}", name=f"xiT{i}_{c}"
                        )
                        if c == 0:
                            nc.vector.tensor_copy(xi[:], ps[:])
                        else:
                            nc.scalar.copy(xi[:], ps[:])
                        xiT[i][c] = xi
                    ps = psT.tile([128, 512], BF16, tag="tp", name="tp")
                    for bt in range(4):
                        nc.tensor.transpose(
                            ps[0:32, bt * 128 : (bt + 1) * 128],
                            deint(xbt[bt][:], i, 256, 32),
                            ident[:],
                        )
                    xi = xtpool.tile([32, BL], BF16, tag="xc2", name=f"xiT{i}_2")
                    nc.vector.tensor_copy(xi[:], ps[0:32, :])
                    xiT[i][2] = xi

                def emit_logits(i):
                    for jt, (j0, jc) in enumerate(CH):
                        pst = psS.tile([128, 512], F32, tag="ps_st", name="ps_st")
                        for lt, (l0, lc) in enumerate(CH):
                            nc.tensor.matmul(
                                pst[0:jc, :],
                                whkT[lt][:, j0 : j0 + jc],
                                xiT[i][lt][:],
                                start=(lt == 0),
                                stop=(lt == 2),
                            )
                        col = i * 3 + jt
                        if jt < 2:
                            eout = EA[i][:, jt * BL : (jt + 1) * BL]
                        else:
                            eout = EC[i][:]
                        nc.scalar.activation(
                            eout,
                            pst[0:jc, :],
                            AF.Exp,
                            bias=shiftc[0:jc, 0:1],
                            scale=SCALE,
                            accum_out=densb[0:jc, col : col + 1],
                        )

                def emit_vt(i):
                    for ntc, (n0, ncnt) in enumerate(CH):
                        pv = psV.tile([128, 512], F32, tag="ps_vt", name="ps_vt")
                        for mt, (m0, mc) in enumerate(CH):
                            nc.tensor.matmul(
                                pv[0:ncnt, :],
                                wv_b[mt][:, n0 : n0 + ncnt],
                                xiT[i][mt][:],
                                start=(mt == 0),
                                stop=(mt == 2),
                            )
                        if ntc < 2:
                            zv = zA[i][:, ntc * BL : (ntc + 1) * BL]
                            ev = EA[i][:, ntc * BL : (ntc + 1) * BL]
                        else:
                            zv = zC[i][:]
                            ev = EC[i][:]
                        nc.vector.tensor_mul(zv, pv[0:ncnt, :], ev)

                emit_transposes(0)
                for i in range(P1):
                    if i + 1 < P1:
                        emit_transposes(i + 1)
                    emit_logits(i)
                    if i >= 2 and i <= 5:  # plw casts mid-FIFO (DMAs done by now)
                        emit_plw_cast(2 * (i - 2))
                        emit_plw_cast(2 * (i - 2) + 1)
                    if i == 3:
                        nc.gpsimd.dma_start(cc_in[0][:], densb[:, 0:12])
                        nc.gpsimd.collective_compute(
                            "AllReduce",
                            ALU.add,
                            replica_groups=[list(range(N_CORES))],
                            ins=[cc_in[0][:]],
                            outs=[cc_out[0][:]],
                        )
                nc.gpsimd.dma_start(cc_in[1][:], densb[:, 12:24])
                nc.gpsimd.collective_compute(
                    "AllReduce",
                    ALU.add,
                    replica_groups=[list(range(N_CORES))],
                    ins=[cc_in[1][:]],
                    outs=[cc_out[1][:]],
                )

                # vT + z-muls fill the AllReduce window
                for i in range(P1):
                    emit_vt(i)

                # plw de-interleave (fills the AllReduce window), per seq-half
                for sh in range(2):
                    for i in range(P1):
                        for c in range(2):
                            j0 = c * 128
                            ps = psT2.tile([128, 512], BF16, tag="tp2", name="tp2")
                            for st in range(4):
                                nc.tensor.transpose(
                                    ps[:, st * 128 : (st + 1) * 128],
                                    deint(pwb[sh * 4 + st][:], i, j0, 128),
                                    ident[:],
                                )
                            dst = rk[i][c][:, sh * 512 : (sh + 1) * 512]
                            if (i + c) % 2 == 0:
                                nc.vector.tensor_copy(dst, ps[:])
                            else:
                                nc.scalar.copy(dst, ps[:])
                        ps = psT2.tile([128, 512], BF16, tag="tp2", name="tp2")
                        for st in range(4):
                            nc.tensor.transpose(
                                ps[0:32, st * 128 : (st + 1) * 128],
                                deint(pwb[sh * 4 + st][:], i, 256, 32),
                                ident[:],
                            )
                        nc.vector.tensor_copy(
                            rk2[i][:, sh * 512 : (sh + 1) * 512],
                            ps[0:32, :],
                        )

            # ---------------- AR consume ----------------
            nc.gpsimd.dma_start(den_all[:, 0:12], cc_out[0][:])
            nc.vector.reciprocal(recip[:, 0:12], den_all[:, 0:12])
            nc.gpsimd.dma_start(den_all[:, 12:24], cc_out[1][:])
            nc.vector.reciprocal(recip[:, 12:24], den_all[:, 12:24])
            den_bf = cpool.tile([128, 24], BF16, tag="den_bf", name="den_bf")
            nc.vector.tensor_copy(den_bf[:], den_all[:])

            # ---------------- scan + projection ----------------
            with (
                tc.tile_pool(name="tmp", bufs=1) as tmppool,
                tc.tile_pool(name="osb", bufs=2) as outpool,
                tc.tile_pool(name="psP", bufs=1, space="PSUM") as psP,
            ):
                ttA = tmppool.tile([128, 2 * BL], BF16, tag="ttA", name="ttA")
                tsA = tmppool.tile([128, 2 * BL], BF16, tag="tsA", name="tsA")
                gA = tmppool.tile([128, 2 * BL], BF16, tag="gA", name="gA")
                tt2 = tmppool.tile([32, BL], BF16, tag="tt2", name="tt2")
                ts2 = tmppool.tile([32, BL], BF16, tag="ts2", name="ts2")
                g2 = tmppool.tile([32, BL], BF16, tag="g2", name="g2")

                # re-warm the PE clock right before the projection: junk
                # matmuls anchored on the AllReduce result (overwritten by
                # the start=True proj_len_b pre-load below)
                wps = psP.tile([128, 512], F32, tag="pj00", name="pj_warm")
                for w in range(16):
                    nc.tensor.matmul(
                        wps[0:24, :], den_bf[:], rk[0][0][:, 0:512],
                        start=True, stop=True,
                    )

                # pre-load proj_len_b into the 8 projection PSUM banks
                pps = {}
                for half in range(2):
                    for bc in range(4):
                        pp = psP.tile(
                            [128, 512], F32, tag=f"pj{half}{bc}", name=f"pj{half}{bc}"
                        )
                        nc.tensor.matmul(
                            pp[:],
                            ones_bf[:],
                            plb_sb[0:1, half * 512 : (half + 1) * 512],
                            start=True,
                            stop=False,
                        )
                        pps[(half, bc)] = pp

                def proj_tile(lsrc, off, rsrc, last):
                    for half in range(2):
                        for bc in range(4):
                            nc.tensor.matmul(
                                pps[(half, bc)][:],
                                lsrc[:, off + bc * 128 : off + (bc + 1) * 128],
                                rsrc[:, half * 512 : (half + 1) * 512],
                                start=False,
                                stop=(last and half == 1 and bc == 3),
                            )

                for i in range(P1):
                    cols = [i * 3, i * 3 + 1, i * 3 + 2]
                    zc = zC[i][:]
                    if i == 0:
                        # y_0 = z_0 * recip
                        for c in range(2):
                            nc.scalar.mul(
                                zA[0][:, c * BL : (c + 1) * BL],
                                zA[0][:, c * BL : (c + 1) * BL],
                                mul=recip[0:128, cols[c] : cols[c] + 1],
                            )
                        nc.scalar.mul(
                            zc, zc, mul=recip[0:32, cols[2] : cols[2] + 1]
                        )
                    else:
                        # g = tanh(a1*y + b1) * sigmoid(a2*y + b2)
                        nc.scalar.activation(
                            ttA[:], zA[i - 1][:], AF.Tanh,
                            bias=bcast[0:128, 2:3], scale=bcast[0:128, 0:1],
                        )
                        nc.scalar.activation(
                            tsA[:], zA[i - 1][:], AF.Sigmoid,
                            bias=bcast[0:128, 3:4], scale=bcast[0:128, 1:2],
                        )
                        nc.scalar.activation(
                            tt2[:], zC[i - 1][:], AF.Tanh,
                            bias=bcast[0:32, 2:3], scale=bcast[0:32, 0:1],
                        )
                        nc.scalar.activation(
                            ts2[:], zC[i - 1][:], AF.Sigmoid,
                            bias=bcast[0:32, 3:4], scale=bcast[0:32, 1:2],
                        )
                        nc.vector.tensor_mul(gA[:], ttA[:], tsA[:])
                        nc.gpsimd.tensor_mul(g2[:], tt2[:], ts2[:])
                        # y_i = z_i * recip + g   (normalize folded in)
                        for c in range(2):
                            zv = zA[i][:, c * BL : (c + 1) * BL]
                            nc.vector.scalar_tensor_tensor(
                                zv, zv,
                                recip[0:128, cols[c] : cols[c] + 1],
                                gA[:, c * BL : (c + 1) * BL],
                                op0=ALU.mult, op1=ALU.add,
                            )
                        nc.vector.scalar_tensor_tensor(
                            zc, zc,
                            recip[0:32, cols[2] : cols[2] + 1],
                            g2[:],
                            op0=ALU.mult, op1=ALU.add,
                        )
                    proj_tile(zA[i], 0, rk[i][0], False)
                    proj_tile(zA[i], BL, rk[i][1], False)
                    proj_tile(zC[i], 0, rk2[i], last=(i == P1 - 1))

                for half in range(2):
                    for bc in range(4):
                        ob = outpool.tile([128, 512], F32, tag="osb", name="osb")
                        nc.scalar.copy(ob[:], pps[(half, bc)][:])
                        nc.sync.dma_start(
                            out[bc * 128 : (bc + 1) * 128, half * 512 : (half + 1) * 512],
                            ob[:],
                        )

    nc.compile()
    return nc


_NC = None


def _get_nc():
    global _NC
    if _NC is None:
        _NC = build()
    return _NC


def run(inputs, trace=False):
    nc = _get_nc()
    rep_keys = [
        "w_k1",
        "w_v1",
        "h1",
        "alpha1",
        "alpha2",
        "beta1",
        "beta2",
        "proj_len_w",
        "proj_len_b",
    ]
    x = np.ascontiguousarray(inputs["x"], dtype=np.float32)
    rep = {k: np.ascontiguousarray(inputs[k], dtype=np.float32) for k in rep_keys}
    in_maps = [
        {"x": x[c * BL : (c + 1) * BL], **rep} for c in range(N_CORES)
    ]
    res = run_bass_kernel_spmd(
        nc, in_maps, core_ids=list(range(N_CORES)), trace=trace
    )
    full = np.concatenate([res.results[c]["out"] for c in range(N_CORES)], axis=0)
    return full, res


def kernel(**inputs):
    full, _ = run(inputs, trace=False)
    return full
